# revision 3
# baseline (speedup 1.0000x reference)
"""Clustered Linformer Attention — TRN2 Bass kernel, batch-parallel over 8 NeuronCores.

v2: bf16 upstream + host-side layout.
Per core (one batch element b):
  Host:  xT = x_b^T (bf16), wq/wk/wv bf16, E/F bf16 pre-shuffled to per-(h,g)
         contiguous DMA blocks, y returned transposed (host un-transposes).
  A:  q^T = wq^T-blocks @ xT-slices ; k,v = xT-blocks^T @ wk/wv   (PE, bf16)
  B:  kp/vp = k/v-blocks^T E/F, col-tiled: the two 64-wide heads of a d-block
      run concurrently in the PE array (tile_position from out base_partition),
      separate PSUM banks per parity; DVE accumulates across n-groups.
  C:  scores^T_h = kp_h^T q_h^T, row-tiled head pairs (K=64); exp via one ACT
      call per head over a 2-bank [128,2,512] PSUM tile (fused 1/sqrt(d)).
  F:  out_raw^T_h = vp2_h @ expT (ones-column extracts softmax row-sums)
  N:  reciprocal + PE broadcast matmul + GpSimd multiply
  G:  y^T = wd^T-blocks @ concat^T (bias per-partition), written bf16
"""
import sys
import numpy as np

for _p in ("/opt/trn_rl_repo", "/root/.axon_site/_ro/trn_rl_repo"):
    if _p not in sys.path:
        sys.path.insert(0, _p)

import ml_dtypes
import concourse.bacc as bacc
import concourse.tile as tile
from concourse import mybir
from concourse.bass_utils import run_bass_kernel_spmd

B, N, D = 8, 4096, 512
H, R = 8, 256
DEP = D // H          # 64
P = 128
NG = 8                # n-groups for phase A/B
GN = N // NG          # 512 rows per group
NS = 8                # n-strips for phase C..G
SN = N // NS          # 512 cols per strip
F32 = mybir.dt.float32
F32R = mybir.dt.float32r
BF16 = mybir.dt.bfloat16
EXPF = mybir.ActivationFunctionType.Exp
BF = ml_dtypes.bfloat16

_cache = {}


def build_program(repeat=1):
    key = ("nc", repeat)
    if key in _cache:
        return _cache[key]
    nc = bacc.Bacc("TRN2", target_bir_lowering=False, debug=False)
    xT = nc.dram_tensor("xT", [D, N], BF16, kind="ExternalInput").ap()
    wq = nc.dram_tensor("wq", [D, D], BF16, kind="ExternalInput").ap()
    wk = nc.dram_tensor("wk", [D, D], BF16, kind="ExternalInput").ap()
    wv = nc.dram_tensor("wv", [D, D], BF16, kind="ExternalInput").ap()
    wd = nc.dram_tensor("wd", [D, D], F32, kind="ExternalInput").ap()
    E = nc.dram_tensor("E", [H, NG, P, 4, R], BF16, kind="ExternalInput").ap()
    Fm = nc.dram_tensor("F", [H, NG, P, 4, R], BF16, kind="ExternalInput").ap()
    ident_in = nc.dram_tensor("ident", [P, P], F32, kind="ExternalInput").ap()
    hb_in = nc.dram_tensor("hb", [P, D], F32, kind="ExternalInput").ap()
    ones_in = nc.dram_tensor("ones", [P, 1], BF16, kind="ExternalInput").ap()
    bT_in = nc.dram_tensor("bT", [P, 4], F32, kind="ExternalInput").ap()
    y = nc.dram_tensor("y", [D, N], BF16, kind="ExternalOutput").ap()

    with tile.TileContext(nc) as tc, nc.allow_low_precision(reason="bf16 kernel"):
      for _rep in range(repeat):
        with tc.tile_pool(name="outer", bufs=1) as po:
            # ---- persistent tiles ----
            qT = [po.tile([P, N], BF16, tag=f"qT{c}", name=f"qT{c}") for c in range(4)]
            kpA = [po.tile([P, R], F32, tag=f"kpA{p}", name=f"kpA{p}") for p in range(4)]
            vpA = [po.tile([P, R], F32, tag=f"vpA{p}", name=f"vpA{p}") for p in range(4)]
            kpS = [po.tile([P, R], BF16, tag=f"kpS{p}", name=f"kpS{p}") for p in range(4)]
            vp2 = [[po.tile([P, P], BF16, tag=f"vp2_{h}_{rc}", name=f"vp2_{h}_{rc}")
                    for rc in range(2)] for h in range(H)]
            bT_t = po.tile([P, 4], F32, tag="bT", name="bT")
            wd_t = [po.tile([P, D], F32R, tag=f"wd{c}", name=f"wd{c}") for c in range(4)]
            ident = po.tile([P, P], F32, tag="ident", name="ident")
            hbr = [po.tile([P, P], F32R, tag=f"hb{p}", name=f"hb{p}") for p in range(4)]
            ones_f = po.tile([P, 1], BF16, tag="ones", name="ones")
            S_t = po.tile([P, SN], F32, tag="S", name="S")
            Sr_t = po.tile([P, SN], F32R, tag="Sr", name="Sr")

            nc.sync.dma_start(ident[:], ident_in)
            nc.sync.dma_start(bT_t[:], bT_in)
            nc.sync.dma_start(ones_f[:], ones_in)
            # rows 8.. of S / Sr never written per-strip; keep them finite
            nc.gpsimd.memset(S_t[:], 1.0)
            nc.gpsimd.memset(Sr_t[:], 0.0)

            # ================= PHASE A+B =================
            with tc.tile_pool(name="pw", bufs=1) as pw, \
                 tc.tile_pool(name="pxs", bufs=6) as pxs, \
                 tc.tile_pool(name="pkv", bufs=8) as pkv, \
                 tc.tile_pool(name="pef", bufs=6) as pef, \
                 tc.tile_pool(name="psA", bufs=3, space="PSUM") as psA, \
                 tc.tile_pool(name="psB", bufs=1, space="PSUM") as psB:

                # constants that need rounding to f32r (stationary use)
                stage = pw.tile([P, D], F32, tag="hbstage", name="hbstage")
                nc.sync.dma_start(stage[:], hb_in)
                for p in range(4):
                    nc.vector.tensor_copy(hbr[p][:], stage[:, p * P:(p + 1) * P])
                for c in range(4):
                    wds = pw.tile([P, D], F32, tag="wdraw", name="wdraw")
                    nc.sync.dma_start(wds[:], wd[c * P:(c + 1) * P, :])
                    nc.vector.tensor_copy(wd_t[c][:], wds[:])

                wq_t = [pw.tile([P, D], BF16, tag=f"wq{c}", name=f"wq{c}") for c in range(4)]
                wk_t = [pw.tile([P, D], BF16, tag=f"wk{c}", name=f"wk{c}") for c in range(4)]
                wv_t = [pw.tile([P, D], BF16, tag=f"wv{c}", name=f"wv{c}") for c in range(4)]
                for c in range(4):
                    nc.sync.dma_start(wq_t[c][:], wq[c * P:(c + 1) * P, :])
                    nc.sync.dma_start(wk_t[c][:], wk[c * P:(c + 1) * P, :])
                    nc.sync.dma_start(wv_t[c][:], wv[c * P:(c + 1) * P, :])

                for g in range(NG):
                    n0 = g * GN
                    # per-group column slices of xT (moving for q, stationary for k/v)
                    xs = []
                    for c in range(4):
                        t = pxs.tile([P, GN], BF16, tag="xs", name="xs")
                        nc.sync.dma_start(t[:], xT[c * P:(c + 1) * P, n0:n0 + GN])
                        xs.append(t)
                    # q^T
                    for dq in range(4):
                        qp = psA.tile([P, GN], F32, tag="qkv", name="qkv")
                        for c in range(4):
                            nc.tensor.matmul(
                                qp[:], wq_t[c][:, dq * P:(dq + 1) * P], xs[c][:],
                                start=(c == 0), stop=(c == 3))
                        nc.scalar.copy(qT[dq][:, n0:n0 + GN], qp[:])
                    # k, v (n-major, bf16 for B-phase stationaries)
                    kg = [pkv.tile([P, D], BF16, tag="kg", name="kg") for i in range(4)]
                    vg = [pkv.tile([P, D], BF16, tag="vg", name="vg") for i in range(4)]
                    for i in range(4):
                        kp_ = psA.tile([P, D], F32, tag="qkv", name="qkv")
                        for c in range(4):
                            nc.tensor.matmul(
                                kp_[:], xs[c][:, i * P:(i + 1) * P], wk_t[c][:],
                                start=(c == 0), stop=(c == 3))
                        nc.scalar.copy(kg[i][:], kp_[:])
                        vp_ = psA.tile([P, D], F32, tag="qkv", name="qkv")
                        for c in range(4):
                            nc.tensor.matmul(
                                vp_[:], xs[c][:, i * P:(i + 1) * P], wv_t[c][:],
                                start=(c == 0), stop=(c == 3))
                        nc.scalar.copy(vg[i][:], vp_[:])
                    # B: project k, v through E_h, F_h. Column-tiled: the two
                    # 64-wide heads of a d-block run concurrently in the PE
                    # array (par0 -> array cols 0-63, par1 -> 64-127), each
                    # accumulating into its own PSUM bank.
                    for pidx in range(4):
                        kpg = [psB.tile([P, SN], F32, tag=f"kpg{par}", name=f"kpg{par}")
                               for par in range(2)]
                        vpg = [psB.tile([P, SN], F32, tag=f"vpg{par}", name=f"vpg{par}")
                               for par in range(2)]
                        for par in range(2):
                            h = 2 * pidx + par
                            ro = DEP * par
                            Eh = pef.tile([P, 4, R], BF16, tag="ef", name="ef")
                            nc.sync.dma_start(Eh[:], E[h, g])
                            Fh = pef.tile([P, 4, R], BF16, tag="ef", name="ef")
                            nc.sync.dma_start(Fh[:], Fm[h, g])
                            cs = slice(pidx * P + ro, pidx * P + ro + DEP)
                            for i in range(4):
                                nc.tensor.matmul(
                                    kpg[par][ro:ro + DEP, 0:R],
                                    kg[i][:, cs], Eh[:, i, :],
                                    start=(i == 0), stop=(i == 3))
                            for i in range(4):
                                nc.tensor.matmul(
                                    vpg[par][ro:ro + DEP, 0:R],
                                    vg[i][:, cs], Fh[:, i, :],
                                    start=(i == 0), stop=(i == 3))
                        for par in range(2):
                            sl = slice(DEP * par, DEP * par + DEP)
                            if g == 0:
                                nc.vector.tensor_copy(kpA[pidx][sl, :], kpg[par][sl, 0:R])
                                nc.vector.tensor_copy(vpA[pidx][sl, :], vpg[par][sl, 0:R])
                            else:
                                nc.vector.tensor_add(
                                    kpA[pidx][sl, :], kpA[pidx][sl, :], kpg[par][sl, 0:R])
                                nc.vector.tensor_add(
                                    vpA[pidx][sl, :], vpA[pidx][sl, :], vpg[par][sl, 0:R])

                # round kp to bf16 stationaries; transpose vp into natural
                # layout with ones-column row-sum extractor
                for p in range(4):
                    nc.vector.tensor_copy(kpS[p][:], kpA[p][:])
                    for rc in range(2):
                        vt = psA.tile([P, P], F32, tag="qkv", name="qkv")
                        nc.tensor.transpose(
                            vt[:], vpA[p][:, rc * P:(rc + 1) * P], ident[:])
                        for par in range(2):
                            h = 2 * p + par
                            ro = DEP * par
                            oro = DEP * (1 - par)
                            nc.vector.tensor_copy(
                                vp2[h][rc][:, ro:ro + DEP], vt[:, ro:ro + DEP])
                            nc.vector.tensor_copy(
                                vp2[h][rc][:, oro:oro + 1], ones_f[:])
                            nc.gpsimd.memset(vp2[h][rc][:, oro + 1:oro + DEP], 0.0)

            # ================= PHASE C..G =================
            with tc.tile_pool(name="pexp", bufs=4) as pexp, \
                 tc.tile_pool(name="pstag", bufs=10) as pstag, \
                 tc.tile_pool(name="pcs", bufs=2) as pcs, \
                 tc.tile_pool(name="pbc", bufs=2) as pbc, \
                 tc.tile_pool(name="psml", bufs=4) as psml, \
                 tc.tile_pool(name="psS", bufs=2, space="PSUM") as psS, \
                 tc.tile_pool(name="psF", bufs=2, space="PSUM") as psF, \
                 tc.tile_pool(name="psY", bufs=2, space="PSUM") as psY:
                for s in range(NS):
                    c0 = s * SN
                    csR = pcs.tile([P, 4, SN], F32R, tag="csR", name="csR")
                    stags = []
                    for c in range(4):  # head pair (2c, 2c+1), row-tiled K=64
                        scp = [psS.tile([P, 2, SN], F32, tag="sc", name="sc")
                               for par in range(2)]
                        for rc in range(2):
                            for par in range(2):
                                rs = slice(DEP * par, DEP * par + DEP)
                                nc.tensor.matmul(
                                    scp[par][:, rc, :],
                                    kpS[c][rs, rc * P:(rc + 1) * P],
                                    qT[c][rs, c0:c0 + SN],
                                    start=True, stop=True)
                        expT = [pexp.tile([P, 2, SN], BF16, tag="expT", name="expT")
                                for par in range(2)]
                        for par in range(2):
                            nc.scalar.activation(
                                expT[par][:], scp[par][:], EXPF,
                                scale=float(1.0 / np.sqrt(np.float32(DEP))))
                        for par in range(2):
                            h = 2 * c + par
                            oro = DEP * (1 - par)
                            fp = psF.tile([P, SN], F32, tag="fo", name="fo")
                            for rc in range(2):
                                nc.tensor.matmul(
                                    fp[:], vp2[h][rc][:], expT[par][:, rc, :],
                                    start=(rc == 0), stop=(rc == 1))
                            stag = pstag.tile([P, SN], F32, tag="stag", name="stag")
                            if par == 0:
                                nc.vector.tensor_copy(stag[:], fp[:])
                            else:
                                nc.scalar.copy(stag[:], fp[:])
                            nc.gpsimd.tensor_copy(S_t[h:h + 1, :], stag[oro:oro + 1, :])
                            stags.append(stag)
                    nc.vector.reciprocal(Sr_t[0:H, :], S_t[0:H, :])
                    for p in range(4):
                        bcp = psS.tile([P, 2, SN], F32, tag="sc", name="sc")
                        nc.tensor.matmul(bcp[:, 0, :], hbr[p][:], Sr_t[:],
                                         start=True, stop=True)
                        bcs = pbc.tile([P, SN], F32, tag="bcs", name="bcs")
                        nc.vector.tensor_copy(bcs[:], bcp[:, 0, :])
                        for par in range(2):
                            h = 2 * p + par
                            ro = DEP * par
                            nc.gpsimd.tensor_mul(
                                csR[ro:ro + DEP, p, :],
                                stags[h][ro:ro + DEP, :],
                                bcs[ro:ro + DEP, :])
                    # y^T = wd^T @ concat^T  (bias is per-partition in this layout)
                    for dq in range(4):
                        yp = psY.tile([P, SN], F32, tag="y", name="y")
                        for c2 in range(4):
                            nc.tensor.matmul(
                                yp[:], wd_t[c2][:, dq * P:(dq + 1) * P],
                                csR[:, c2, :],
                                start=(c2 == 0), stop=(c2 == 3))
                        ys = psml.tile([P, SN], BF16, tag="ysb", name="ysb")
                        if dq % 2 == 0:
                            nc.vector.tensor_scalar_add(ys[:], yp[:], bT_t[:, dq:dq + 1])
                        else:
                            nc.scalar.activation(
                                ys[:], yp[:], mybir.ActivationFunctionType.Copy,
                                bias=bT_t[:, dq:dq + 1])
                        nc.sync.dma_start(y[dq * P:(dq + 1) * P, c0:c0 + SN], ys[:])

    nc.compile()
    _cache[key] = nc
    return nc


def make_in_maps(x, wq, wk, wv, E, F, w_dense, b_dense):
    x = np.asarray(x, dtype=np.float32)
    E = np.asarray(E, np.float32)
    F = np.asarray(F, np.float32)
    # per-(head, group) contiguous DMA blocks: [H, NG, P, 4, R]
    esh = lambda t: np.ascontiguousarray(
        t.reshape(H, NG, 4, P, R).transpose(0, 1, 3, 2, 4).astype(BF))
    consts = {
        "wq": np.ascontiguousarray(np.asarray(wq, np.float32).astype(BF)),
        "wk": np.ascontiguousarray(np.asarray(wk, np.float32).astype(BF)),
        "wv": np.ascontiguousarray(np.asarray(wv, np.float32).astype(BF)),
        "wd": np.ascontiguousarray(np.asarray(w_dense, np.float32)),
        "E": esh(E),
        "F": esh(F),
        "ident": np.eye(P, dtype=np.float32),
        "hb": _make_hb(),
        "ones": np.ones((P, 1), dtype=BF),
        "bT": np.ascontiguousarray(
            np.asarray(b_dense, np.float32).reshape(4, P).T),
    }
    return [{"xT": np.ascontiguousarray(x[b].T.astype(BF)), **consts}
            for b in range(B)]


def _make_hb():
    hb = np.zeros((P, D), dtype=np.float32)
    for p in range(4):
        for m in range(P):
            hb[2 * p + m // DEP, p * P + m] = 1.0
    return hb


def kernel(x, wq, wk, wv, E, F, w_dense, b_dense):
    nc = build_program()
    in_maps = make_in_maps(x, wq, wk, wv, E, F, w_dense, b_dense)
    res = run_bass_kernel_spmd(nc, in_maps, list(range(B)))
    out = np.stack(
        [np.asarray(res.results[b]["y"]).astype(np.float32).T for b in range(B)],
        axis=0)
    return np.ascontiguousarray(out)


def postprocess(y_stack):
    """bench2 helper: [B, D, N] bf16 -> [B, N, D] f32."""
    return np.ascontiguousarray(
        np.asarray(y_stack).astype(np.float32).transpose(0, 2, 1))
